# revision 33
# baseline (speedup 1.0000x reference)
"""AWQ (4-bit group-quantized) linear layer on 8 Trainium2 NeuronCores.

Computation: out = inputs @ dequant(qweight, qzeros, scales) + bias
  inputs  [M, K]  f32
  qweight [K, N/8] int32 (AWQ-packed 8x int4 per word, interleaved order)
  qzeros  [G, N/8] int32 (same packing), scales [G, N] f32, bias [N] f32
  out     [M, N]  f32        (M=K=4096, N=11008, G=32, group_size=128)

Sharding: column-parallel (out_features) across 8 cores; inputs replicated.
Each core dequantizes its W shard on-device (DVE byte ops -> nibbles ->
*scale - zp*scale, bf16) and runs a bf16 matmul with f32 PSUM accumulation;
bias added on the way out of PSUM.

AWQ nibble order: AWQ_REVERSE_ORDER = [0,4,1,5,2,6,3,7], i.e. output column
n = 8p + j comes from nibble position order[j] of packed word p. In byte
terms (little-endian int32): byte index (4p + 2*b1 + b0)'s lo nibble maps to
output column 8p + 4*b0 + b1, its hi nibble to 8p + 4*b0 + b1 + 2. Both maps
are affine in (p, b0, b1), so two strided DVE ops (AND 0xF / LSR 4) unpack a
whole [128, NSH] group tile.

Loop structure: m-tiles are processed in interleaved PAIRS (two k-loops in
flight over 6 PSUM banks) so that while the PE chases the group-by-group
dequantization at kernel start, its stall gaps stay below the ~3.4us HAM
idle window that would re-throttle the PE clock to 1.2 GHz.
"""

import numpy as np

_NC = 8
_GS = 128  # AWQ group size (= one 128-row k-tile per group)


def _build(M, K, NSH):
    """Build the single-core Bass module for an [M,K] x [K,NSH] AWQ matmul."""
    import concourse.mybir as mybir
    import concourse.tile as tile
    from concourse import bacc

    f32 = mybir.dt.float32
    bf16 = mybir.dt.bfloat16
    u8 = mybir.dt.uint8
    u16 = mybir.dt.uint16
    Alu = mybir.AluOpType

    assert M % 256 == 0 and K % 128 == 0 and NSH % 8 == 0
    G = K // _GS
    KT = K // 128
    MT = M // 128
    PB = NSH // 4  # packed uint16 halfwords per row of the shard (NSH/8 int32)

    ntiles = []
    n0 = 0
    while n0 < NSH:
        ns = min(512, NSH - n0)
        ntiles.append((n0, ns))
        n0 += ns
    NT = len(ntiles)

    nc = bacc.Bacc()
    xT = nc.dram_tensor("xT", [K, M], f32, kind="ExternalInput")
    qw = nc.dram_tensor("qw", [K, PB], u16, kind="ExternalInput")
    qz = nc.dram_tensor("qz", [G, PB], u16, kind="ExternalInput")
    sc = nc.dram_tensor("scales", [G, NSH], f32, kind="ExternalInput")
    bi = nc.dram_tensor("bias", [1, NSH], f32, kind="ExternalInput")
    out = nc.dram_tensor("out", [M, NSH], f32, kind="ExternalOutput")
    zs_dram = nc.dram_tensor("zs_scratch", [G, NSH], bf16)  # Internal
    sb_dram = nc.dram_tensor("sc_bf16_scratch", [G, NSH], bf16)  # Internal

    def unpack_nibbles(eng, dst_ap, src_u16_ap):
        # uint16 halfword view of the AWQ packing: halfword w = 2p + c of a
        # row holds nibble positions p' = 0..3 (shift 4*p'), which map to
        # output column n = 8p + 2*p' + c. Every op keeps a packed step-1
        # count-2 innermost dim on both sides, so the DVE runs them in
        # 2x_1P mode. dst must be uint16 (bitVec ops cannot dtype-cast).
        q_in = src_u16_ap.rearrange("k (p c) -> k p c", c=2)
        n_out = dst_ap.rearrange("k (p x) -> k p x", x=8)
        for pp in range(4):
            eng.tensor_scalar(
                n_out[:, :, 2 * pp : 2 * pp + 2],
                q_in,
                4 * pp,
                0xF,
                Alu.logical_shift_right,
                Alu.bitwise_and,
            )

    with tile.TileContext(nc) as tc:
        with (
            tc.tile_pool(name="singles", bufs=1) as singles,
            tc.tile_pool(name="wpool", bufs=G) as wpool,
            tc.tile_pool(name="qwp", bufs=4) as qwp,
            tc.tile_pool(name="bcp", bufs=3) as bcp,
            tc.tile_pool(name="nibp", bufs=2) as nibp,
            tc.tile_pool(name="xfp", bufs=2) as xfp,
            tc.tile_pool(name="xbp", bufs=3) as xbp,
            tc.tile_pool(name="outp", bufs=3) as outp,
        ):
            # ---- zs = zp * scale (bf16) + bf16 scales, staged to DRAM for
            # per-group partition-broadcast reads. The whole dequant data
            # path lives on the GpSimd DMA queue: FIFO order makes the
            # scratch-write -> broadcast-read chain safe, and it keeps the
            # small dequant transfers from queueing behind the big x slabs
            # on the Sync queue.
            sc_sb = singles.tile([G, NSH], f32)
            nc.gpsimd.dma_start(sc_sb[:], sc[:])
            qz_sb = singles.tile([G, PB], u16)
            nc.gpsimd.dma_start(qz_sb[:], qz[:])
            zp_sb = singles.tile([G, NSH], u16)
            unpack_nibbles(nc.vector, zp_sb, qz_sb)
            zs_sb = singles.tile([G, NSH], bf16)
            nc.vector.tensor_tensor(zs_sb[:], zp_sb[:], sc_sb[:], Alu.mult)
            nc.gpsimd.dma_start(zs_dram[:], zs_sb[:])
            scb16_sb = singles.tile([G, NSH], bf16)
            nc.vector.tensor_copy(scb16_sb[:], sc_sb[:])
            nc.gpsimd.dma_start(sb_dram[:], scb16_sb[:])

            # ---- x slab loader for an m-tile PAIR:
            # xT[:, mp*128:(mp+2)*128] -> bf16 [128, KT, 256] (1KB DMA rows)
            KH = KT // 4 if KT % 4 == 0 else KT

            def load_xb(mp):
                xb = xbp.tile([128, KT, 256], bf16, tag="xb", name=f"xb_{mp}")
                for h0 in range(0, KT, KH):
                    xf = xfp.tile([128, KH, 256], f32, tag="xf", name=f"xf_{mp}_{h0}")
                    src = xT[
                        h0 * 128 : (h0 + KH) * 128, mp * 128 : (mp + 2) * 128
                    ].rearrange("(kt p) m -> p kt m", p=128)
                    nc.sync.dma_start(xf[:], src)
                    nc.scalar.copy(xb[:, h0 : h0 + KH, :], xf[:])
                return xb

            # prefetch the first pair's activations before dequant kicks off
            xb_next = load_xb(0)

            # ---- dequantize W shard into SBUF, one bf16 tile per group
            w_tiles = []
            for g in range(G):
                qw_sb = qwp.tile([128, PB], u16, tag="qw", name=f"qw_{g}")
                nc.gpsimd.dma_start(qw_sb[:], qw[g * 128 : (g + 1) * 128, :])
                scb = bcp.tile([128, NSH], bf16, tag="scb", name=f"scb_{g}")
                nc.scalar.dma_start(
                    scb[:], sb_dram[g : g + 1, :].to_broadcast((128, NSH))
                )
                zsb = bcp.tile([128, NSH], bf16, tag="zsb", name=f"zsb_{g}")
                nc.gpsimd.dma_start(
                    zsb[:], zs_dram[g : g + 1, :].to_broadcast((128, NSH))
                )
                nib = nibp.tile([128, NSH], u16, tag="nib", name=f"nib_{g}")
                unpack_nibbles(nc.vector, nib, qw_sb)
                t = nibp.tile([128, NSH], bf16, tag="tmp", name=f"t_{g}")
                nc.vector.tensor_tensor(t[:], nib[:], scb[:], Alu.mult)
                wt = wpool.tile([128, NSH], bf16, tag="w", name=f"w_{g}")
                nc.vector.tensor_tensor(wt[:], t[:], zsb[:], Alu.subtract)
                w_tiles.append(wt)

            # ---- bias broadcast (first needed at the first PSUM drain,
            # so keep it off the critical dequant DMA path)
            bias_bc = singles.tile([128, NSH], f32)
            nc.scalar.dma_start(bias_bc[:], bi[:].to_broadcast((128, NSH)))

            # ---- main matmul
            def drain(mi, pt, n0, ns, name):
                ob = outp.tile([128, 512], f32, tag="ob", name=f"ob_{name}")
                nc.vector.tensor_tensor(
                    ob[:, :ns], pt[:, :ns], bias_bc[:, n0 : n0 + ns], Alu.add
                )
                nc.sync.dma_start(
                    out[mi * 128 : (mi + 1) * 128, n0 : n0 + ns], ob[:, :ns]
                )

            # While the PE is rate-limited by the group-by-group dequant
            # ("the chase"), give it 4 m-tiles of work instead of 2 by
            # spending all 8 PSUM banks on n-columns [0, 1024) of m-tiles
            # 0..3 and deferring their last n-tile to a post-chase pass.
            use_quad = MT % 4 == 0 and MT >= 4 and NT == 3
            start_mp = 4 if use_quad else 0
            if use_quad:
                xbq = [xb_next, load_xb(2)]
                with tc.tile_pool(name="quadp", bufs=1, space="PSUM") as quadp:
                    psq = [
                        [
                            quadp.tile(
                                [128, 512], f32, tag=f"cq{q}{ti}", name=f"cq_{q}_{ti}"
                            )
                            for ti in range(2)
                        ]
                        for q in range(4)
                    ]
                    for kt in range(KT):
                        for q in range(4):
                            for ti in range(2):
                                n0, ns = ntiles[ti]
                                nc.tensor.matmul(
                                    psq[q][ti][:, :ns],
                                    xbq[q // 2][
                                        :, kt, (q % 2) * 128 : (q % 2) * 128 + 128
                                    ],
                                    w_tiles[kt][:, n0 : n0 + ns],
                                    start=(kt == 0),
                                    stop=(kt == KT - 1),
                                )
                    for q in range(4):
                        for ti in range(2):
                            n0, ns = ntiles[ti]
                            drain(q, psq[q][ti], n0, ns, f"q{q}_{ti}")

            with tc.tile_pool(name="psump", bufs=4, space="PSUM") as psump:
                if use_quad:
                    # deferred last n-tile of m-tiles 0..3 (W fully resident)
                    n0, ns = ntiles[2]
                    for jp in range(2):
                        pst = [
                            psump.tile(
                                [128, 512], f32, tag=f"ps{j}", name=f"pst_{jp}_{j}"
                            )
                            for j in range(2)
                        ]
                        for kt in range(KT):
                            for j in range(2):
                                nc.tensor.matmul(
                                    pst[j][:, :ns],
                                    xbq[jp][:, kt, j * 128 : (j + 1) * 128],
                                    w_tiles[kt][:, n0 : n0 + ns],
                                    start=(kt == 0),
                                    stop=(kt == KT - 1),
                                )
                        for j in range(2):
                            drain(2 * jp + j, pst[j], n0, ns, f"t{jp}_{j}")
                    if start_mp < MT:
                        xb_next = load_xb(start_mp)

                for mp in range(start_mp, MT, 2):
                    mis = (mp, mp + 1)
                    xb = xb_next
                    if mp + 2 < MT:
                        xb_next = load_xb(mp + 2)
                    psums = [
                        [
                            psump.tile(
                                [128, 512], f32, tag=f"ps{j}", name=f"ps_{mp}_{j}_{ti}"
                            )
                            for ti in range(NT)
                        ]
                        for j in range(2)
                    ]
                    for kt in range(KT):
                        for j in range(2):
                            for ti, (n0, ns) in enumerate(ntiles):
                                nc.tensor.matmul(
                                    psums[j][ti][:, :ns],
                                    xb[:, kt, j * 128 : (j + 1) * 128],
                                    w_tiles[kt][:, n0 : n0 + ns],
                                    start=(kt == 0),
                                    stop=(kt == KT - 1),
                                )
                    for j in range(2):
                        for ti, (n0, ns) in enumerate(ntiles):
                            drain(mis[j], psums[j][ti], n0, ns, f"{mp}_{j}_{ti}")

    nc.compile()
    return nc


def make_in_maps(inputs, qweight, qzeros, scales, bias, n_cores=_NC):
    """Shard host inputs column-parallel; inputs (transposed) replicated."""
    NF = scales.shape[1]
    NSH = NF // n_cores
    PS = NSH // 8
    xT = np.ascontiguousarray(inputs.T)
    in_maps = []
    for c in range(n_cores):
        qw_s = np.ascontiguousarray(qweight[:, c * PS : (c + 1) * PS]).view(np.uint16)
        qz_s = np.ascontiguousarray(qzeros[:, c * PS : (c + 1) * PS]).view(np.uint16)
        sc_s = np.ascontiguousarray(scales[:, c * NSH : (c + 1) * NSH])
        bi_s = np.ascontiguousarray(bias[c * NSH : (c + 1) * NSH]).reshape(1, NSH)
        in_maps.append(
            {"xT": xT, "qw": qw_s, "qz": qz_s, "scales": sc_s, "bias": bi_s}
        )
    return in_maps


_nc_cache = {}


def _get_nc(M, K, NSH):
    key = (M, K, NSH)
    if key not in _nc_cache:
        _nc_cache[key] = _build(M, K, NSH)
    return _nc_cache[key]


def kernel(inputs, qweight, qzeros, scales, bias):
    from concourse.bass_utils import run_bass_kernel_spmd

    M, K = inputs.shape
    NF = scales.shape[1]
    NSH = NF // _NC
    nc = _get_nc(M, K, NSH)
    in_maps = make_in_maps(inputs, qweight, qzeros, scales, bias)
    res = run_bass_kernel_spmd(nc, in_maps, core_ids=list(range(_NC)))
    return np.concatenate([r["out"] for r in res.results], axis=1)


# revision 37
# speedup vs baseline: 1.0304x; 1.0304x over previous
"""AWQ (4-bit group-quantized) linear layer on 8 Trainium2 NeuronCores.

Computation: out = inputs @ dequant(qweight, qzeros, scales) + bias
  inputs  [M, K]  f32
  qweight [K, N/8] int32 (AWQ-packed 8x int4 per word, interleaved order)
  qzeros  [G, N/8] int32 (same packing), scales [G, N] f32, bias [N] f32
  out     [M, N]  f32        (M=K=4096, N=11008, G=32, group_size=128)

Sharding: column-parallel (out_features) across 8 cores; inputs replicated.
Each core dequantizes its W shard on-device (DVE byte ops -> nibbles ->
*scale - zp*scale, bf16) and runs a bf16 matmul with f32 PSUM accumulation;
bias added on the way out of PSUM.

AWQ nibble order: AWQ_REVERSE_ORDER = [0,4,1,5,2,6,3,7], i.e. output column
n = 8p + j comes from nibble position order[j] of packed word p. In byte
terms (little-endian int32): byte index (4p + 2*b1 + b0)'s lo nibble maps to
output column 8p + 4*b0 + b1, its hi nibble to 8p + 4*b0 + b1 + 2. Both maps
are affine in (p, b0, b1), so two strided DVE ops (AND 0xF / LSR 4) unpack a
whole [128, NSH] group tile.

Loop structure: m-tiles are processed in interleaved PAIRS (two k-loops in
flight over 6 PSUM banks) so that while the PE chases the group-by-group
dequantization at kernel start, its stall gaps stay below the ~3.4us HAM
idle window that would re-throttle the PE clock to 1.2 GHz.
"""

import numpy as np

_NC = 8
_GS = 128  # AWQ group size (= one 128-row k-tile per group)


def _build(M, K, NSH):
    """Build the single-core Bass module for an [M,K] x [K,NSH] AWQ matmul."""
    import concourse.mybir as mybir
    import concourse.tile as tile
    from concourse import bacc

    f32 = mybir.dt.float32
    bf16 = mybir.dt.bfloat16
    u8 = mybir.dt.uint8
    u16 = mybir.dt.uint16
    Alu = mybir.AluOpType

    assert M % 256 == 0 and K % 128 == 0 and NSH % 8 == 0
    G = K // _GS
    KT = K // 128
    MT = M // 128
    PB = NSH // 4  # packed uint16 halfwords per row of the shard (NSH/8 int32)

    ntiles = []
    n0 = 0
    while n0 < NSH:
        ns = min(512, NSH - n0)
        ntiles.append((n0, ns))
        n0 += ns
    NT = len(ntiles)

    nc = bacc.Bacc()
    xT = nc.dram_tensor("xT", [K, M], f32, kind="ExternalInput")
    qw = nc.dram_tensor("qw", [K, PB], u16, kind="ExternalInput")
    qz = nc.dram_tensor("qz", [G, PB], u16, kind="ExternalInput")
    sc = nc.dram_tensor("scales", [G, NSH], f32, kind="ExternalInput")
    bi = nc.dram_tensor("bias", [1, NSH], f32, kind="ExternalInput")
    out = nc.dram_tensor("out", [M, NSH], f32, kind="ExternalOutput")
    zs_dram = nc.dram_tensor("zs_scratch", [G, NSH], bf16)  # Internal
    sb_dram = nc.dram_tensor("sc_bf16_scratch", [G, NSH], bf16)  # Internal

    def unpack_nibbles(eng, dst_ap, src_u16_ap):
        # uint16 halfword view of the AWQ packing: halfword w = 2p + c of a
        # row holds nibble positions p' = 0..3 (shift 4*p'), which map to
        # output column n = 8p + 2*p' + c. Every op keeps a packed step-1
        # count-2 innermost dim on both sides, so the DVE runs them in
        # 2x_1P mode. dst must be uint16 (bitVec ops cannot dtype-cast).
        q_in = src_u16_ap.rearrange("k (p c) -> k p c", c=2)
        n_out = dst_ap.rearrange("k (p x) -> k p x", x=8)
        for pp in range(4):
            eng.tensor_scalar(
                n_out[:, :, 2 * pp : 2 * pp + 2],
                q_in,
                4 * pp,
                0xF,
                Alu.logical_shift_right,
                Alu.bitwise_and,
            )

    with tile.TileContext(nc) as tc:
        with (
            tc.tile_pool(name="singles", bufs=1) as singles,
            tc.tile_pool(name="wpool", bufs=G) as wpool,
            tc.tile_pool(name="qwp", bufs=4) as qwp,
            tc.tile_pool(name="bcp", bufs=3) as bcp,
            tc.tile_pool(name="nibp", bufs=2) as nibp,
            tc.tile_pool(name="xfp", bufs=2) as xfp,
            tc.tile_pool(name="xbp", bufs=2) as xbp,
            tc.tile_pool(name="outp", bufs=3) as outp,
            tc.tile_pool(name="psump", bufs=4, space="PSUM") as psump,
        ):
            # ---- zs = zp * scale (bf16) + bf16 scales, staged to DRAM for
            # per-group partition-broadcast reads. The whole dequant data
            # path lives on the GpSimd DMA queue: FIFO order makes the
            # scratch-write -> broadcast-read chain safe, and it keeps the
            # small dequant transfers from queueing behind the big x slabs
            # on the Sync queue.
            sc_sb = singles.tile([G, NSH], f32)
            nc.gpsimd.dma_start(sc_sb[:], sc[:])
            qz_sb = singles.tile([G, PB], u16)
            nc.gpsimd.dma_start(qz_sb[:], qz[:])
            zp_sb = singles.tile([G, NSH], u16)
            unpack_nibbles(nc.vector, zp_sb, qz_sb)
            zs_sb = singles.tile([G, NSH], bf16)
            nc.vector.tensor_tensor(zs_sb[:], zp_sb[:], sc_sb[:], Alu.mult)
            nc.gpsimd.dma_start(zs_dram[:], zs_sb[:])
            scb16_sb = singles.tile([G, NSH], bf16)
            nc.vector.tensor_copy(scb16_sb[:], sc_sb[:])
            nc.gpsimd.dma_start(sb_dram[:], scb16_sb[:])

            # ---- x slab loader for an m-tile PAIR:
            # xT[:, mp*128:(mp+2)*128] -> bf16 [128, KT, 256] (1KB DMA rows)
            KH = KT // 4 if KT % 4 == 0 else KT

            def load_xb(mp):
                xb = xbp.tile([128, KT, 256], bf16, tag="xb", name=f"xb_{mp}")
                for h0 in range(0, KT, KH):
                    xf = xfp.tile([128, KH, 256], f32, tag="xf", name=f"xf_{mp}_{h0}")
                    src = xT[
                        h0 * 128 : (h0 + KH) * 128, mp * 128 : (mp + 2) * 128
                    ].rearrange("(kt p) m -> p kt m", p=128)
                    nc.sync.dma_start(xf[:], src)
                    nc.scalar.copy(xb[:, h0 : h0 + KH, :], xf[:])
                return xb

            # prefetch the first two pairs' activations before dequant kicks
            # off, so their ScalarE casts are scheduled ahead of the scb
            # broadcast descriptors (whose pool-slot waits would otherwise
            # block the casts inside the ACT instruction stream).
            xb_cur = load_xb(0)
            xb_nxt = load_xb(2) if MT > 2 else None

            # ---- dequantize W shard into SBUF, one bf16 tile per group.
            # qw loads are emitted two groups ahead so their descriptors sit
            # in front of the zsb slot-waits in the GpSimd stream.
            def load_qw(g):
                qw_sb = qwp.tile([128, PB], u16, tag="qw", name=f"qw_{g}")
                nc.gpsimd.dma_start(qw_sb[:], qw[g * 128 : (g + 1) * 128, :])
                return qw_sb

            qw_tiles = {g: load_qw(g) for g in range(min(2, G))}
            w_tiles = []
            for g in range(G):
                if g + 2 < G:
                    qw_tiles[g + 2] = load_qw(g + 2)
                qw_sb = qw_tiles.pop(g)
                scb = bcp.tile([128, NSH], bf16, tag="scb", name=f"scb_{g}")
                nc.scalar.dma_start(
                    scb[:], sb_dram[g : g + 1, :].to_broadcast((128, NSH))
                )
                zsb = bcp.tile([128, NSH], bf16, tag="zsb", name=f"zsb_{g}")
                nc.gpsimd.dma_start(
                    zsb[:], zs_dram[g : g + 1, :].to_broadcast((128, NSH))
                )
                nib = nibp.tile([128, NSH], u16, tag="nib", name=f"nib_{g}")
                unpack_nibbles(nc.vector, nib, qw_sb)
                t = nibp.tile([128, NSH], bf16, tag="tmp", name=f"t_{g}")
                nc.vector.tensor_tensor(t[:], nib[:], scb[:], Alu.mult)
                wt = wpool.tile([128, NSH], bf16, tag="w", name=f"w_{g}")
                nc.vector.tensor_tensor(wt[:], t[:], zsb[:], Alu.subtract)
                w_tiles.append(wt)

            # ---- bias broadcast (first needed at the first PSUM drain,
            # so keep it off the critical dequant DMA path)
            bias_bc = singles.tile([128, NSH], f32)
            nc.scalar.dma_start(bias_bc[:], bi[:].to_broadcast((128, NSH)))

            # ---- main matmul: m-tiles in interleaved pairs. The next pair
            # slab is prefetched AFTER this pair's k-loop so chase-phase
            # casts keep scheduling priority.
            for mp in range(0, MT, 2):
                mis = (mp, mp + 1)
                xb = xb_cur
                psums = [
                    [
                        psump.tile([128, 512], f32, tag=f"ps{j}", name=f"ps_{mp}_{j}_{ti}")
                        for ti in range(NT)
                    ]
                    for j in range(2)
                ]
                for kt in range(KT):
                    for j in range(2):
                        for ti, (n0, ns) in enumerate(ntiles):
                            nc.tensor.matmul(
                                psums[j][ti][:, :ns],
                                xb[:, kt, j * 128 : (j + 1) * 128],
                                w_tiles[kt][:, n0 : n0 + ns],
                                start=(kt == 0),
                                stop=(kt == KT - 1),
                            )
                xb_cur = xb_nxt
                if mp + 4 < MT:
                    xb_nxt = load_xb(mp + 4)
                for j in range(2):
                    for ti, (n0, ns) in enumerate(ntiles):
                        ob = outp.tile([128, 512], f32, tag="ob", name=f"ob_{mp}_{j}_{ti}")
                        nc.vector.tensor_tensor(
                            ob[:, :ns],
                            psums[j][ti][:, :ns],
                            bias_bc[:, n0 : n0 + ns],
                            Alu.add,
                        )
                        nc.sync.dma_start(
                            out[mis[j] * 128 : (mis[j] + 1) * 128, n0 : n0 + ns],
                            ob[:, :ns],
                        )

    nc.compile()
    return nc


def make_in_maps(inputs, qweight, qzeros, scales, bias, n_cores=_NC):
    """Shard host inputs column-parallel; inputs (transposed) replicated."""
    NF = scales.shape[1]
    NSH = NF // n_cores
    PS = NSH // 8
    xT = np.ascontiguousarray(inputs.T)
    in_maps = []
    for c in range(n_cores):
        qw_s = np.ascontiguousarray(qweight[:, c * PS : (c + 1) * PS]).view(np.uint16)
        qz_s = np.ascontiguousarray(qzeros[:, c * PS : (c + 1) * PS]).view(np.uint16)
        sc_s = np.ascontiguousarray(scales[:, c * NSH : (c + 1) * NSH])
        bi_s = np.ascontiguousarray(bias[c * NSH : (c + 1) * NSH]).reshape(1, NSH)
        in_maps.append(
            {"xT": xT, "qw": qw_s, "qz": qz_s, "scales": sc_s, "bias": bi_s}
        )
    return in_maps


_nc_cache = {}


def _get_nc(M, K, NSH):
    key = (M, K, NSH)
    if key not in _nc_cache:
        _nc_cache[key] = _build(M, K, NSH)
    return _nc_cache[key]


def kernel(inputs, qweight, qzeros, scales, bias):
    from concourse.bass_utils import run_bass_kernel_spmd

    M, K = inputs.shape
    NF = scales.shape[1]
    NSH = NF // _NC
    nc = _get_nc(M, K, NSH)
    in_maps = make_in_maps(inputs, qweight, qzeros, scales, bias)
    res = run_bass_kernel_spmd(nc, in_maps, core_ids=list(range(_NC)))
    return np.concatenate([r["out"] for r in res.results], axis=1)
